# revision 14
# baseline (speedup 1.0000x reference)
"""GCN GraphConv (norm='both') on 8 Trainium2 NeuronCores.

Strategy (V5, replicated projection + gated gather overlap):
  - Output rows (dst nodes) sharded across 8 cores: core c owns rows
    [c*6250, (c+1)*6250), viewed as 49 blocks of 128 dst rows.
  - Projection phase REPLICATED: every core computes the full
    h = (feat @ W) * outdeg^-1/2 table (50176 nodes, bf16, 256B rows in
    rho layout row = (n%128)*392 + n//128) into local DRAM, in 28
    superblocks of 14 chunks.  Each superblock's h-write DMA bumps a
    semaphore.  No collective, no cross-core sync at all.
  - Edge phase: per-core edges grouped by dst block; chunks of 128 edge
    slots, budgets (CL/CH per block) shared across cores so the program
    is SPMD-uniform.  Within each (block, lo/hi kind) the edges are
    sorted by source chunk, so each gather chunk has a monotone
    "gate" = number of h superblocks it needs.  Chunks are assigned to
    NSEG segments by cross-core-max gate; the gather stream waits on the
    h semaphore once per segment and otherwise runs concurrently with
    the projection phase (dma_gather's DRAM source is not
    dependency-tracked, so the manual sem is the only ordering).
  - Each chunk: dma_gather pulls h[rho(src)] rows into SBUF token
    layout; one-hot S[e,d] = (dstval[e] == d) built on DVE in bf16
    (batches of 8 chunks per instruction, physically-replicated iota to
    avoid broadcast APs); PE accumulates psum[128 dst, 64] += S.T @ G
    per chunk; per (block, segment) the psum partial is flushed into an
    SBUF f32 accumulator.
  - int16 gather-index limit handled by a lo/hi split at SPLIT=32768
    with a base-offset view h_all[SPLIT:].
  - Final per block: scale acc by indeg^-1/2, add bias, one dense DMA
    out (host un-permutes the [p, b] row order).

Host does integer-only graph preprocessing (degree counts, edge
bucketing, index remapping, gate computation).  bf16 is used for
feat/W/h (kernel-internal precision choice); accumulation is fp32 in
PSUM / SBUF.
"""

import sys

sys.path.insert(0, "/opt/trn_rl_repo")

import numpy as np
import ml_dtypes

import concourse.bacc as bacc
import concourse.bass as bass
import concourse.mybir as mybir
import concourse.tile as tile
from concourse.bass_utils import run_bass_kernel_spmd

F32 = mybir.dt.float32
BF16 = mybir.dt.bfloat16
I16 = mybir.dt.int16
P = 128
NO_MATCH = 999.0  # dstval for pad slots; never equals iota 0..127

N_NODES = 50000
N_FEAT = 256
N_OUT = 64


def _cfg_full():
    return dict(
        NPAD=50176,          # padded node count (392 chunks of 128)
        NCHUNK=392,
        SUP=14,              # h chunks per projection superblock
        NSUP=28,
        OWN=6250,
        OWN_PAD=6272,        # 49 blocks of 128
        SPLIT=32768,
        BLK_G=7,             # dst blocks per group
        SEG=7,               # gather chunks per dma_gather instruction
        WS=8,                # one-hot chunks per DVE instruction
        GATES=(6, 12, 18, 24, 28),   # segment gate boundaries (superblocks)
        NF=N_FEAT,
        NO=N_OUT,
        NUM_DEV=8,
        CL=None,             # per-block lo chunk budgets (len 49)
        CH=None,             # per-block hi chunk budgets
        SEGOF=None,          # (b, kind, j) -> segment index
    )


def _edge_layout(cfg):
    """Shared (cross-core) processing order bookkeeping.

    Returns:
      order: list over segments of list over groups of
             list of (b, kind, j, stream_pos) in matmul order
             (block-major, lo chunks then hi chunks per block)
      lo_counts/hi_counts: per (seg, grp): number of lo / hi chunks
    """
    CL, CH, SEGOF = cfg["CL"], cfg["CH"], cfg["SEGOF"]
    NBLK = cfg["OWN_PAD"] // P
    NSEG = len(cfg["GATES"])
    BLK_G = cfg["BLK_G"]
    groups = [(g0, min(g0 + BLK_G, NBLK)) for g0 in range(0, NBLK, BLK_G)]
    order = []
    for s in range(NSEG):
        seg_groups = []
        for g0, g1 in groups:
            lo_pos = 0
            hi_pos = 0
            chunks = []
            for b in range(g0, g1):
                for j in range(CL[b]):
                    if SEGOF[(b, 0, j)] == s:
                        chunks.append((b, 0, j, lo_pos))
                        lo_pos += 1
                for j in range(CH[b]):
                    if SEGOF[(b, 1, j)] == s:
                        chunks.append((b, 1, j, hi_pos))
                        hi_pos += 1
            # reorder stream positions: lo chunks take 0..lo_pos-1 in
            # (b, j) order; hi likewise.  chunks[] already iterates in
            # that order so the pos values above are correct.
            seg_groups.append((chunks, lo_pos, hi_pos))
        order.append(seg_groups)
    return order, groups


def build_nc(cfg, debug=False):
    NPAD, NCHUNK = cfg["NPAD"], cfg["NCHUNK"]
    SUP, NSUP = cfg["SUP"], cfg["NSUP"]
    OWN_PAD, SPLIT = cfg["OWN_PAD"], cfg["SPLIT"]
    SEG, WS = cfg["SEG"], cfg["WS"]
    GATES = cfg["GATES"]
    NF, NO = cfg["NF"], cfg["NO"]
    CL, CH = cfg["CL"], cfg["CH"]
    NBLK = OWN_PAD // P
    KC = NF // P
    NSEG = len(GATES)
    assert SUP * NSUP == NCHUNK

    nc = bacc.Bacc(
        "TRN2",
        target_bir_lowering=False,
        debug=debug,
        num_devices=cfg["NUM_DEV"],
        num_swdge_queues=4,
        dynamic_dma_scratch_size=32768,
    )

    TL, TH = sum(CL) * P, sum(CH) * P
    TOTCK = sum(CL) + sum(CH)

    featT = nc.dram_tensor("featT", [NF, NPAD], BF16, kind="ExternalInput")
    weight = nc.dram_tensor("weight", [NF, NO], BF16, kind="ExternalInput")
    bias_t = nc.dram_tensor("bias_t", [P, NO], F32, kind="ExternalInput")
    odeg = nc.dram_tensor("odeg", [P, NCHUNK], F32, kind="ExternalInput")
    ideg = nc.dram_tensor("ideg", [P, NBLK], F32, kind="ExternalInput")
    iota_h = nc.dram_tensor("iota_h", [P, WS * P], BF16, kind="ExternalInput")
    gidxL = nc.dram_tensor("gidxL", [P, max(TL // 16, 1)], I16, kind="ExternalInput")
    gidxH = nc.dram_tensor("gidxH", [P, max(TH // 16, 1)], I16, kind="ExternalInput")
    dvalsT = nc.dram_tensor("dvalsT", [P, TOTCK], BF16, kind="ExternalInput")

    out = nc.dram_tensor("out", [P, NBLK * NO], F32, kind="ExternalOutput")

    h_all = nc.dram_tensor("h_all", [NPAD, P], BF16)
    # rho layout: row = (n % 128) * NCHUNK + n // 128 -> [p, chunk, 128]
    h_view = h_all.ap().rearrange("(p c) d -> p c d", p=P)

    h_sem = nc.alloc_semaphore("h_ready")
    h_tok = nc.alloc_sbuf_tensor("h_tok", [P, NSUP * 2], BF16)
    h_tok2 = nc.alloc_sbuf_tensor("h_tok2", [P, NSUP * 2], BF16)

    order, groups = _edge_layout(cfg)

    with tile.TileContext(nc) as tc:
        with (
            tc.tile_pool(name="const", bufs=1) as cpool,
            tc.tile_pool(name="feat", bufs=2) as fpool,
            tc.tile_pool(name="hstage", bufs=2) as hpool,
            tc.tile_pool(name="psA", bufs=2, space="PSUM") as ppoolA,
            tc.tile_pool(name="psB", bufs=2, space="PSUM") as ppoolB,
            tc.tile_pool(name="gath", bufs=24) as gpool,
            tc.tile_pool(name="idx", bufs=3) as ipool,
            tc.tile_pool(name="onehot", bufs=4) as spool,
            tc.tile_pool(name="fin", bufs=1) as finpool,
        ):
            # ---- constants ----
            w_sb = []
            for k in range(KC):
                wk = cpool.tile([P, NO], BF16, tag=f"w{k}")
                nc.sync.dma_start(wk[:], weight[k * P:(k + 1) * P, :])
                w_sb.append(wk)
            bias_sb = cpool.tile([P, NO], F32, tag="bias")
            nc.sync.dma_start(bias_sb[:], bias_t[:])
            iota_sb = cpool.tile([P, WS * P], BF16, tag="iota")
            nc.sync.dma_start(iota_sb[:], iota_h[:])

            osc = cpool.tile([P, NCHUNK], F32, tag="osc")
            nc.sync.dma_start(osc[:], odeg[:])
            nc.vector.tensor_scalar_max(osc[:], osc[:], 1.0)
            nc.scalar.activation(osc[:], osc[:], mybir.ActivationFunctionType.Sqrt)
            nc.vector.reciprocal(osc[:], osc[:])

            isc = cpool.tile([P, NBLK], F32, tag="isc")
            nc.sync.dma_start(isc[:], ideg[:])
            nc.vector.tensor_scalar_max(isc[:], isc[:], 1.0)
            nc.scalar.activation(isc[:], isc[:], mybir.ActivationFunctionType.Sqrt)
            nc.vector.reciprocal(isc[:], isc[:])

            # f32 accumulator for all 49 blocks; memset before gathers start
            acc = finpool.tile([P, NBLK * NO], F32, tag="acc")
            nc.vector.memset(acc[:], 0.0)

            # ---- phase 1: replicated h = (feat @ W) * outdeg^-1/2 ----
            for s in range(NSUP):
                fts = []
                for k in range(KC):
                    ft = fpool.tile([P, SUP * P], BF16, tag=f"ft{k}")
                    nc.sync.dma_start(
                        ft[:],
                        featT[k * P:(k + 1) * P, s * SUP * P:(s + 1) * SUP * P],
                    )
                    fts.append(ft)
                hst = hpool.tile([P, SUP * P], BF16, tag="hst")
                # junk cols NO..P are never read downstream, but the
                # h-write DMA reads the whole tile (sim init check)
                nc.vector.memset(
                    hst[:].rearrange("p (c d) -> p c d", d=P)[:, :, NO:], 0.0
                )
                for cc in range(SUP):
                    hp = ppoolA.tile([P, NO], F32, tag="hp")
                    for k in range(KC):
                        nc.tensor.matmul(
                            hp[:],
                            fts[k][:, cc * P:(cc + 1) * P],
                            w_sb[k][:],
                            start=(k == 0),
                            stop=(k == KC - 1),
                        )
                    c = s * SUP + cc
                    # scalar engine: hst = Copy(hp * osc[:, c])  (bf16 out)
                    nc.scalar.activation(
                        hst[:, cc * P:cc * P + NO],
                        hp[:],
                        mybir.ActivationFunctionType.Copy,
                        scale=osc[:, c:c + 1],
                    )
                # pad cols NO..P of each row slot are junk (never read)
                nc.sync.dma_start(h_view[:, s * SUP:(s + 1) * SUP, :], hst[:])
                # token read-back: the h-write above IS range-tracked for
                # regular DMAs, so this tiny read waits for it; a vector
                # copy of the token then bumps h_sem (DMA instructions
                # have no free sem-update slot under TileContext).
                nc.sync.dma_start(
                    h_tok[:, s * 2:(s + 1) * 2],
                    h_view[:, (s + 1) * SUP - 1:(s + 1) * SUP, 0:2].rearrange(
                        "p o d -> p (o d)"
                    ),
                )
                nc.vector.tensor_copy(
                    h_tok2[:, s * 2:(s + 1) * 2], h_tok[:, s * 2:(s + 1) * 2]
                ).then_inc(h_sem, 16)

            # ---- edge phase: gated gathers + one-hot matmul reduce ----
            h_full = h_all.ap()
            h_hi = h_all.ap()[SPLIT:, :]

            # per-(seg, grp, kind) stream offsets into gidxL/gidxH/dvals
            offL = 0
            offH = 0
            offD = 0
            qcnt = [0]
            maxLo = max(
                (nlo for seg in order for (_, nlo, _) in seg), default=1
            )
            maxHi = max(
                (nhi for seg in order for (_, _, nhi) in seg), default=1
            )
            maxCk = max(
                (len(ch) for seg in order for (ch, _, _) in seg), default=1
            )

            for s_i in range(NSEG):
                nc.gpsimd.wait_ge(h_sem, 16 * GATES[s_i])
                for gi, (g0, g1) in enumerate(groups):
                    chunks, nlo, nhi = order[s_i][gi]
                    if not chunks:
                        continue
                    # idx loads for this (seg, grp)
                    gixL = gixH = None
                    if nlo:
                        gixL = ipool.tile([P, max(maxLo * 8, 8)], I16, tag="gixL")
                        nc.sync.dma_start(
                            gixL[:, : nlo * 8],
                            gidxL[:, offL * 8: (offL + nlo) * 8],
                        )
                    if nhi:
                        gixH = ipool.tile([P, max(maxHi * 8, 8)], I16, tag="gixH")
                        nc.sync.dma_start(
                            gixH[:, : nhi * 8],
                            gidxH[:, offH * 8: (offH + nhi) * 8],
                        )
                    nck = len(chunks)
                    dv = ipool.tile([P, max(maxCk, 1)], BF16, tag="dv")
                    nc.sync.dma_start(dv[:, :nck], dvalsT[:, offD: offD + nck])

                    # gathers: per kind, split into <=SEG chunk instructions
                    tiles = ([], [])
                    for kind, (ck, gix, base_ap) in enumerate(
                        [(nlo, gixL, h_full), (nhi, gixH, h_hi)]
                    ):
                        for s0 in range(0, ck, SEG):
                            n = min(SEG, ck - s0)
                            gt = gpool.tile([P, SEG, P], BF16, tag="gt")
                            nc.gpsimd.dma_gather(
                                gt[:, :n, :],
                                base_ap,
                                gix[:, s0 * 8:(s0 + n) * 8],
                                n * P,
                                n * P,
                                P,
                                queue_num=qcnt[0] % 4,
                            )
                            qcnt[0] += 1
                            tiles[kind].append(gt)

                    # one-hot batches of WS chunks + per-chunk matmuls
                    # chunks[] is in matmul order; build S lazily per batch
                    sw_tiles = {}
                    for w0 in range(0, nck, WS):
                        wn = min(WS, nck - w0)
                        Sw = spool.tile([P, WS * P], BF16, tag="S")
                        nc.vector.tensor_tensor(
                            Sw[:, : wn * P].rearrange("p (w d) -> p w d", d=P),
                            iota_sb[:, : wn * P].rearrange("p (w d) -> p w d", d=P),
                            dv[:, w0:w0 + wn].rearrange(
                                "p (w o) -> p w o", o=1
                            ).broadcast_to([P, wn, P]),
                            op=mybir.AluOpType.is_equal,
                        )
                        sw_tiles[w0 // WS] = Sw

                    # matmuls: iterate blocks of this group in order
                    ci = 0
                    b_cur = -1
                    pb = None
                    while ci < nck:
                        b = chunks[ci][0]
                        # find extent of this block's chunks in this seg
                        cj = ci
                        while cj < nck and chunks[cj][0] == b:
                            cj += 1
                        pb = ppoolB.tile([P, NO], F32, tag="pb")
                        for t in range(ci, cj):
                            _, kind, j, pos = chunks[t]
                            gt = tiles[kind][pos // SEG]
                            Sw = sw_tiles[t // WS]
                            nc.tensor.matmul(
                                pb[:],
                                Sw[:, (t % WS) * P:(t % WS + 1) * P],
                                gt[:, pos % SEG, :NO],
                                start=(t == ci),
                                stop=(t == cj - 1),
                            )
                        osl = slice(b * NO, (b + 1) * NO)
                        nc.vector.tensor_tensor(
                            acc[:, osl], acc[:, osl], pb[:],
                            op=mybir.AluOpType.add,
                        )
                        ci = cj

                    offL += nlo
                    offH += nhi
                    offD += nck

            # ---- final: scale by indeg^-1/2, add bias, write out ----
            ot = finpool.tile([P, NBLK * NO], F32, tag="out")
            for b in range(NBLK):
                osl = slice(b * NO, (b + 1) * NO)
                nc.vector.tensor_scalar_mul(ot[:, osl], acc[:, osl], isc[:, b:b + 1])
            nc.vector.tensor_tensor(
                ot[:].rearrange("p (b d) -> p b d", d=NO),
                ot[:].rearrange("p (b d) -> p b d", d=NO),
                bias_sb[:].rearrange("p (o d) -> p o d", o=1).broadcast_to(
                    [P, NBLK, NO]
                ),
                op=mybir.AluOpType.add,
            )
            nc.sync.dma_start(out.ap(), ot[:])

    nc.compile()
    return nc


def _wrap_stream(a):
    """flat [L] int array (L % 16 == 0) -> [128, L//16] int16 wrapped+replicated."""
    L = len(a)
    if L == 0:
        return np.zeros((P, 1), np.int16)
    w = a.reshape(L // 16, 16).T  # [16, L//16]
    return np.ascontiguousarray(np.tile(w, (8, 1)).astype(np.int16))


def _prep_host(feat, weight, bias, src, dst, cfg):
    NPAD, NCHUNK, SUP = cfg["NPAD"], cfg["NCHUNK"], cfg["SUP"]
    OWN, OWN_PAD, SPLIT = cfg["OWN"], cfg["OWN_PAD"], cfg["SPLIT"]
    GATES = cfg["GATES"]
    NF, NO, WS = cfg["NF"], cfg["NO"], cfg["WS"]
    NBLK = OWN_PAD // P
    n = feat.shape[0]
    ncore = cfg["NUM_DEV"]

    src = np.asarray(src)
    dst = np.asarray(dst)
    # rho: node u -> h row (u%128)*NCHUNK + u//128
    schunk = src // P               # source chunk 0..391 (gating)
    rho_src = (src % P).astype(np.int64) * NCHUNK + schunk
    ssup = schunk // SUP            # superblock 0..27

    outdeg = np.bincount(src, minlength=NPAD).astype(np.float32)
    outdeg[n:] = 1.0
    indeg = np.bincount(dst, minlength=ncore * OWN).astype(np.float32)

    featT_full = np.zeros((NF, NPAD), ml_dtypes.bfloat16)
    featT_full[:, :n] = np.asarray(feat, np.float32).T

    bias_t = np.ascontiguousarray(
        np.tile(np.asarray(bias, np.float32)[None, :], (P, 1))
    )
    weight_b = np.ascontiguousarray(
        np.asarray(weight, np.float32).astype(ml_dtypes.bfloat16)
    )
    iota_h = np.ascontiguousarray(
        np.tile(np.arange(P, dtype=np.float32)[None, :], (P, WS)).astype(
            ml_dtypes.bfloat16
        )
    )
    odeg_r = np.ascontiguousarray(outdeg.reshape(NCHUNK, P).T)

    core_of = dst // OWN
    # buckets[c][b] = (g_lo, d_lo, sup_lo, g_hi, d_hi, sup_hi), each sorted
    # by source chunk
    buckets = []
    for c in range(ncore):
        msk = core_of == c
        g = rho_src[msk]
        sup_e = ssup[msk]
        dl = (dst[msk] - c * OWN).astype(np.int64)
        blk = dl // P
        d128 = dl % P
        lo = g < SPLIT
        per_blk = []
        for b in range(NBLK):
            mb = blk == b
            mbl = mb & lo
            mbh = mb & ~lo
            ol = np.argsort(sup_e[mbl], kind="stable")
            oh = np.argsort(sup_e[mbh], kind="stable")
            per_blk.append(
                (
                    g[mbl][ol], d128[mbl][ol], sup_e[mbl][ol],
                    g[mbh][oh] - SPLIT, d128[mbh][oh], sup_e[mbh][oh],
                )
            )
        buckets.append(per_blk)

    CL = [0] * NBLK
    CH = [0] * NBLK
    for c in range(ncore):
        for b in range(NBLK):
            gl, _, _, gh, _, _ = buckets[c][b]
            CL[b] = max(CL[b], (len(gl) + P - 1) // P)
            CH[b] = max(CH[b], (len(gh) + P - 1) // P)

    # shared chunk gates: gate[b][k][j] = max over cores of (chunk max sup)+1
    gate = {}
    for b in range(NBLK):
        for k, CX in ((0, CL), (1, CH)):
            for j in range(CX[b]):
                gate[(b, k, j)] = 1  # pad chunks read row 0 (sup 0)
    for c in range(ncore):
        for b in range(NBLK):
            gl, _, sl, gh, _, sh = buckets[c][b]
            for k, sups in ((0, sl), (1, sh)):
                for j in range((len(sups) + P - 1) // P):
                    mx = int(sups[j * P: (j + 1) * P].max()) + 1
                    key = (b, k, j)
                    if mx > gate[key]:
                        gate[key] = mx

    # segment assignment by shared gate
    SEGOF = {}
    for key, gv in gate.items():
        for s_i, gb in enumerate(GATES):
            if gv <= gb:
                SEGOF[key] = s_i
                break
    cfg["CL"], cfg["CH"], cfg["SEGOF"] = CL, CH, SEGOF

    order, _ = _edge_layout(cfg)

    # per-core streams in processing order
    in_maps = []
    for c in range(ncore):
        # chunk-indexed views of this core's edges
        def chunk_data(b, k, j):
            gl, dvl, _, gh, dvh, _ = buckets[c][b]
            g, dvv = (gl, dvl) if k == 0 else (gh, dvh)
            gs = g[j * P:(j + 1) * P]
            ds = dvv[j * P:(j + 1) * P]
            # pad rows must be written by superblock 0 (gate 1): lo pad
            # -> row 0 (node 0); hi pad -> row 84*392 = 32928 (node 84),
            # the first hi-range row with source chunk 0.
            pad_idx = 0 if k == 0 else (84 * NCHUNK - SPLIT)
            gpad = np.full(P, pad_idx, np.int64)
            dpad = np.full(P, NO_MATCH, np.float32)
            gpad[: len(gs)] = gs
            dpad[: len(ds)] = ds
            return gpad, dpad

        gl_parts = []
        gh_parts = []
        dv_parts = []
        for s_i in range(len(GATES)):
            for (chunks, nlo, nhi) in order[s_i]:
                lo_chunks = [t for t in chunks if t[1] == 0]
                hi_chunks = [t for t in chunks if t[1] == 1]
                for b, k, j, pos in lo_chunks:
                    gp, _ = chunk_data(b, 0, j)
                    gl_parts.append(gp)
                for b, k, j, pos in hi_chunks:
                    gp, _ = chunk_data(b, 1, j)
                    gh_parts.append(gp)
                for b, k, j, pos in chunks:
                    _, dp = chunk_data(b, k, j)
                    dv_parts.append(dp)

        gl_stream = (
            np.concatenate(gl_parts) if gl_parts else np.zeros(0, np.int64)
        )
        gh_stream = (
            np.concatenate(gh_parts) if gh_parts else np.zeros(0, np.int64)
        )
        dvals = (
            np.stack(dv_parts) if dv_parts else np.zeros((0, P), np.float32)
        )

        ideg_c = np.full(OWN_PAD, 1.0, np.float32)
        ideg_c[:OWN] = indeg[c * OWN:(c + 1) * OWN]

        in_maps.append(
            {
                "featT": featT_full,
                "weight": weight_b,
                "bias_t": bias_t,
                "odeg": odeg_r,
                "ideg": np.ascontiguousarray(ideg_c.reshape(NBLK, P).T),
                "iota_h": iota_h,
                "gidxL": _wrap_stream(gl_stream),
                "gidxH": _wrap_stream(gh_stream),
                "dvalsT": np.ascontiguousarray(
                    dvals.T.astype(ml_dtypes.bfloat16)
                ),
            }
        )
    return in_maps


_NC_CACHE = {}


def _get_nc(cfg):
    key = (tuple(cfg["CL"]), tuple(cfg["CH"]),
           tuple(sorted(cfg["SEGOF"].items())))
    if key not in _NC_CACHE:
        _NC_CACHE[key] = build_nc(cfg)
    return _NC_CACHE[key]


def kernel(feat, weight, bias, src, dst, _trace=False, _trace_kwargs=None):
    cfg = _cfg_full()
    in_maps = _prep_host(feat, weight, bias, src, dst, cfg)
    nc = _get_nc(cfg)
    res = run_bass_kernel_spmd(
        nc,
        in_maps,
        core_ids=list(range(cfg["NUM_DEV"])),
        trace=_trace,
        **(_trace_kwargs or {}),
    )
    OWN, NBLK, NO = cfg["OWN"], cfg["OWN_PAD"] // P, cfg["NO"]
    outs = []
    for c in range(cfg["NUM_DEV"]):
        arr = res.results[c]["out"].reshape(P, NBLK, NO)
        own = arr.transpose(1, 0, 2).reshape(NBLK * P, NO)[:OWN]
        outs.append(own)
    out = np.ascontiguousarray(np.concatenate(outs, axis=0).astype(np.float32))
    if _trace:
        return out, res
    return out


# revision 19
# speedup vs baseline: 1.0848x; 1.0848x over previous
"""GCN GraphConv (norm='both') on 8 Trainium2 NeuronCores.

Strategy (V5, replicated projection + gated gather overlap):
  - Output rows (dst nodes) sharded across 8 cores: core c owns rows
    [c*6250, (c+1)*6250), viewed as 49 blocks of 128 dst rows.
  - Projection phase REPLICATED: every core computes the full
    h = (feat @ W) * outdeg^-1/2 table (50176 nodes, bf16, 256B rows in
    rho layout row = (n%128)*392 + n//128) into local DRAM, in 28
    superblocks of 14 chunks.  Each superblock's h-write DMA bumps a
    semaphore.  No collective, no cross-core sync at all.
  - Edge phase: per-core edges grouped by dst block; chunks of 128 edge
    slots, budgets (CL/CH per block) shared across cores so the program
    is SPMD-uniform.  Within each (block, lo/hi kind) the edges are
    sorted by source chunk, so each gather chunk has a monotone
    "gate" = number of h superblocks it needs.  Chunks are assigned to
    NSEG segments by cross-core-max gate; the gather stream waits on the
    h semaphore once per segment and otherwise runs concurrently with
    the projection phase (dma_gather's DRAM source is not
    dependency-tracked, so the manual sem is the only ordering).
  - Each chunk: dma_gather pulls h[rho(src)] rows into SBUF token
    layout; one-hot S[e,d] = (dstval[e] == d) built on DVE in bf16
    (batches of 8 chunks per instruction, physically-replicated iota to
    avoid broadcast APs); PE accumulates psum[128 dst, 64] += S.T @ G
    per chunk; per (block, segment) the psum partial is flushed into an
    SBUF f32 accumulator.
  - int16 gather-index limit handled by a lo/hi split at SPLIT=32768
    with a base-offset view h_all[SPLIT:].
  - Final per block: scale acc by indeg^-1/2, add bias, one dense DMA
    out (host un-permutes the [p, b] row order).

Host does integer-only graph preprocessing (degree counts, edge
bucketing, index remapping, gate computation).  bf16 is used for
feat/W/h (kernel-internal precision choice); accumulation is fp32 in
PSUM / SBUF.
"""

import sys

sys.path.insert(0, "/opt/trn_rl_repo")

import numpy as np
import ml_dtypes

import concourse.bacc as bacc
import concourse.bass as bass
import concourse.mybir as mybir
import concourse.tile as tile
from concourse.bass_utils import run_bass_kernel_spmd

F32 = mybir.dt.float32
BF16 = mybir.dt.bfloat16
I16 = mybir.dt.int16
P = 128
NO_MATCH = 999.0  # dstval for pad slots; never equals iota 0..127

N_NODES = 50000
N_FEAT = 256
N_OUT = 64


def _cfg_full():
    return dict(
        NPAD=50176,          # padded node count (392 chunks of 128)
        NCHUNK=392,
        SUP=14,              # h chunks per projection superblock
        NSUP=28,
        OWN=6250,
        OWN_PAD=6272,        # 49 blocks of 128
        SPLIT=32768,
        BLK_G=7,             # dst blocks per group
        SEG=7,               # gather chunks per dma_gather instruction
        WS=8,                # one-hot chunks per DVE instruction
        GATES=(6, 12, 18, 24, 28),   # segment gate boundaries (superblocks)
        NF=N_FEAT,
        NO=N_OUT,
        NUM_DEV=8,
        CL=None,             # per-block lo chunk budgets (len 49)
        CH=None,             # per-block hi chunk budgets
        SEGOF=None,          # (b, kind, j) -> segment index
    )


def _edge_layout(cfg):
    """Shared (cross-core) processing order bookkeeping.

    Returns:
      order: list over segments of list over groups of
             list of (b, kind, j, stream_pos) in matmul order
             (block-major, lo chunks then hi chunks per block)
      lo_counts/hi_counts: per (seg, grp): number of lo / hi chunks
    """
    CL, CH, SEGOF = cfg["CL"], cfg["CH"], cfg["SEGOF"]
    NBLK = cfg["OWN_PAD"] // P
    NSEG = len(cfg["GATES"])
    BLK_G = cfg["BLK_G"]
    groups = [(g0, min(g0 + BLK_G, NBLK)) for g0 in range(0, NBLK, BLK_G)]
    order = []
    for s in range(NSEG):
        seg_groups = []
        for g0, g1 in groups:
            lo_pos = 0
            hi_pos = 0
            chunks = []
            for b in range(g0, g1):
                for j in range(CL[b]):
                    if SEGOF[(b, 0, j)] == s:
                        chunks.append((b, 0, j, lo_pos))
                        lo_pos += 1
                for j in range(CH[b]):
                    if SEGOF[(b, 1, j)] == s:
                        chunks.append((b, 1, j, hi_pos))
                        hi_pos += 1
            # reorder stream positions: lo chunks take 0..lo_pos-1 in
            # (b, j) order; hi likewise.  chunks[] already iterates in
            # that order so the pos values above are correct.
            seg_groups.append((chunks, lo_pos, hi_pos))
        order.append(seg_groups)
    return order, groups


def build_nc(cfg, debug=False):
    NPAD, NCHUNK = cfg["NPAD"], cfg["NCHUNK"]
    SUP, NSUP = cfg["SUP"], cfg["NSUP"]
    OWN_PAD, SPLIT = cfg["OWN_PAD"], cfg["SPLIT"]
    SEG, WS = cfg["SEG"], cfg["WS"]
    GATES = cfg["GATES"]
    NF, NO = cfg["NF"], cfg["NO"]
    CL, CH = cfg["CL"], cfg["CH"]
    NBLK = OWN_PAD // P
    KC = NF // P
    NSEG = len(GATES)
    assert SUP * NSUP == NCHUNK

    nc = bacc.Bacc(
        "TRN2",
        target_bir_lowering=False,
        debug=debug,
        num_devices=cfg["NUM_DEV"],
        num_swdge_queues=4,
        dynamic_dma_scratch_size=32768,
    )

    TL, TH = sum(CL) * P, sum(CH) * P
    TOTCK = sum(CL) + sum(CH)

    featT = nc.dram_tensor("featT", [NF, NPAD], BF16, kind="ExternalInput")
    weight = nc.dram_tensor("weight", [NF, NO], BF16, kind="ExternalInput")
    bias_t = nc.dram_tensor("bias_t", [P, NO], F32, kind="ExternalInput")
    odeg = nc.dram_tensor("odeg", [P, NCHUNK], F32, kind="ExternalInput")
    ideg = nc.dram_tensor("ideg", [P, NBLK], F32, kind="ExternalInput")
    iota_h = nc.dram_tensor("iota_h", [P, WS * P], BF16, kind="ExternalInput")
    gidxL = nc.dram_tensor("gidxL", [P, max(TL // 16, 1)], I16, kind="ExternalInput")
    gidxH = nc.dram_tensor("gidxH", [P, max(TH // 16, 1)], I16, kind="ExternalInput")
    dvalsT = nc.dram_tensor("dvalsT", [P, TOTCK], BF16, kind="ExternalInput")

    out = nc.dram_tensor("out", [P, NBLK * NO], F32, kind="ExternalOutput")

    h_all = nc.dram_tensor("h_all", [NPAD, P], BF16)
    # rho layout: row = (n % 128) * NCHUNK + n // 128 -> [p, chunk, 128]
    h_view = h_all.ap().rearrange("(p c) d -> p c d", p=P)

    h_sem = nc.alloc_semaphore("h_ready")
    h_tok = nc.alloc_sbuf_tensor("h_tok", [P, NSUP * 2], BF16)
    h_tok2 = nc.alloc_sbuf_tensor("h_tok2", [P, NSUP * 2], BF16)

    order, groups = _edge_layout(cfg)

    with tile.TileContext(nc) as tc:
        with (
            tc.tile_pool(name="const", bufs=1) as cpool,
            tc.tile_pool(name="feat", bufs=3) as fpool,
            tc.tile_pool(name="hstage", bufs=2) as hpool,
            tc.tile_pool(name="psA", bufs=2, space="PSUM") as ppoolA,
            tc.tile_pool(name="psB", bufs=2, space="PSUM") as ppoolB,
            tc.tile_pool(name="gath", bufs=24) as gpool,
            tc.tile_pool(name="onehot", bufs=4) as spool,
            tc.tile_pool(name="fin", bufs=1) as finpool,
        ):
            # ---- constants ----
            w_sb = []
            for k in range(KC):
                wk = cpool.tile([P, NO], BF16, tag=f"w{k}")
                nc.sync.dma_start(wk[:], weight[k * P:(k + 1) * P, :])
                w_sb.append(wk)
            bias_sb = cpool.tile([P, NO], F32, tag="bias")
            nc.sync.dma_start(bias_sb[:], bias_t[:])
            iota_sb = cpool.tile([P, WS * P], BF16, tag="iota")
            nc.sync.dma_start(iota_sb[:], iota_h[:])

            osc = cpool.tile([P, NCHUNK], F32, tag="osc")
            nc.sync.dma_start(osc[:], odeg[:])
            nc.vector.tensor_scalar_max(osc[:], osc[:], 1.0)
            nc.scalar.activation(osc[:], osc[:], mybir.ActivationFunctionType.Sqrt)
            nc.vector.reciprocal(osc[:], osc[:])

            isc = cpool.tile([P, NBLK], F32, tag="isc")
            nc.sync.dma_start(isc[:], ideg[:])
            nc.vector.tensor_scalar_max(isc[:], isc[:], 1.0)
            nc.scalar.activation(isc[:], isc[:], mybir.ActivationFunctionType.Sqrt)
            nc.vector.reciprocal(isc[:], isc[:])

            # f32 accumulator for all 49 blocks; memset before gathers start
            acc = finpool.tile([P, NBLK * NO], F32, tag="acc")
            nc.vector.memset(acc[:], 0.0)

            # all gather indices + dst values loaded upfront into static
            # SBUF (no pool rotation -> no DMA-queue head blocking)
            gixA = cpool.tile([P, max(TL // 16, 1)], I16, tag="gixA")
            nc.sync.dma_start(gixA[:], gidxL[:])
            gixB = cpool.tile([P, max(TH // 16, 1)], I16, tag="gixB")
            nc.sync.dma_start(gixB[:], gidxH[:])
            dv_all = cpool.tile([P, TOTCK], BF16, tag="dva")
            nc.sync.dma_start(dv_all[:], dvalsT[:])

            # ---- phase 1: replicated h = (feat @ W) * outdeg^-1/2 ----
            GW = 7  # chunks per wide psum group (448 f32 < 1 psum bank)
            assert SUP % GW == 0
            for s in range(NSUP):
                fts = []
                for k in range(KC):
                    ft = fpool.tile([P, SUP * P], BF16, tag=f"ft{k}")
                    nc.sync.dma_start(
                        ft[:],
                        featT[k * P:(k + 1) * P, s * SUP * P:(s + 1) * SUP * P],
                    )
                    fts.append(ft)
                hst = hpool.tile([P, SUP * P], BF16, tag="hst")
                hst3 = hst[:].rearrange("p (c d) -> p c d", d=P)
                # junk cols NO..P are never read downstream, but the
                # h-write DMA reads the whole tile (sim init check)
                nc.vector.memset(hst3[:, :, NO:], 0.0)
                for g0 in range(0, SUP, GW):
                    hp = ppoolA.tile([P, GW * NO], F32, tag="hp")
                    for cc in range(GW):
                        for k in range(KC):
                            nc.tensor.matmul(
                                hp[:, cc * NO:(cc + 1) * NO],
                                fts[k][:, (g0 + cc) * P:(g0 + cc + 1) * P],
                                w_sb[k][:],
                                start=(k == 0),
                                stop=(k == KC - 1),
                            )
                    c0 = s * SUP + g0
                    # one batched DVE op scales GW chunks at once
                    nc.vector.tensor_tensor(
                        hst3[:, g0:g0 + GW, :NO],
                        hp[:].rearrange("p (c d) -> p c d", d=NO),
                        osc[:, c0:c0 + GW].rearrange(
                            "p (c o) -> p c o", o=1
                        ).broadcast_to([P, GW, NO]),
                        op=mybir.AluOpType.mult,
                    )
                # h-write + token go on the Activation engine's HWDGE
                # queue so their waits never block the featT loads on the
                # sync queue.
                nc.scalar.dma_start(h_view[:, s * SUP:(s + 1) * SUP, :], hst[:])
                # token read-back: the h-write above IS range-tracked for
                # regular DMAs, so this tiny read waits for it; a vector
                # copy of the token then bumps h_sem (DMA instructions
                # have no free sem-update slot under TileContext).
                nc.scalar.dma_start(
                    h_tok[:, s * 2:(s + 1) * 2],
                    h_view[:, (s + 1) * SUP - 1:(s + 1) * SUP, 0:2].rearrange(
                        "p o d -> p (o d)"
                    ),
                )
                nc.vector.tensor_copy(
                    h_tok2[:, s * 2:(s + 1) * 2], h_tok[:, s * 2:(s + 1) * 2]
                ).then_inc(h_sem, 16)

            # ---- edge phase: gated gathers + one-hot matmul reduce ----
            h_full = h_all.ap()
            h_hi = h_all.ap()[SPLIT:, :]

            # per-(seg, grp, kind) stream offsets into gidxL/gidxH/dvals
            offL = 0
            offH = 0
            offD = 0
            qcnt = [0]

            for s_i in range(NSEG):
                nc.gpsimd.wait_ge(h_sem, 16 * GATES[s_i])
                for gi, (g0, g1) in enumerate(groups):
                    chunks, nlo, nhi = order[s_i][gi]
                    if not chunks:
                        continue
                    nck = len(chunks)

                    # gathers: per kind, split into <=SEG chunk instructions
                    tiles = ([], [])
                    for kind, (ck, gix, off, base_ap) in enumerate(
                        [(nlo, gixA, offL, h_full), (nhi, gixB, offH, h_hi)]
                    ):
                        for s0 in range(0, ck, SEG):
                            n = min(SEG, ck - s0)
                            gt = gpool.tile([P, SEG, P], BF16, tag="gt")
                            nc.gpsimd.dma_gather(
                                gt[:, :n, :],
                                base_ap,
                                gix[:, (off + s0) * 8:(off + s0 + n) * 8],
                                n * P,
                                n * P,
                                P,
                                queue_num=qcnt[0] % 4,
                            )
                            qcnt[0] += 1
                            tiles[kind].append(gt)

                    # one-hot batches of WS chunks + per-chunk matmuls
                    # chunks[] is in matmul order; build S lazily per batch
                    sw_tiles = {}
                    for w0 in range(0, nck, WS):
                        wn = min(WS, nck - w0)
                        Sw = spool.tile([P, WS * P], BF16, tag="S")
                        nc.vector.tensor_tensor(
                            Sw[:, : wn * P].rearrange("p (w d) -> p w d", d=P),
                            iota_sb[:, : wn * P].rearrange("p (w d) -> p w d", d=P),
                            dv_all[:, offD + w0: offD + w0 + wn].rearrange(
                                "p (w o) -> p w o", o=1
                            ).broadcast_to([P, wn, P]),
                            op=mybir.AluOpType.is_equal,
                        )
                        sw_tiles[w0 // WS] = Sw

                    # matmuls: iterate blocks of this group in order
                    ci = 0
                    b_cur = -1
                    pb = None
                    while ci < nck:
                        b = chunks[ci][0]
                        # find extent of this block's chunks in this seg
                        cj = ci
                        while cj < nck and chunks[cj][0] == b:
                            cj += 1
                        pb = ppoolB.tile([P, NO], F32, tag="pb")
                        for t in range(ci, cj):
                            _, kind, j, pos = chunks[t]
                            gt = tiles[kind][pos // SEG]
                            Sw = sw_tiles[t // WS]
                            nc.tensor.matmul(
                                pb[:],
                                Sw[:, (t % WS) * P:(t % WS + 1) * P],
                                gt[:, pos % SEG, :NO],
                                start=(t == ci),
                                stop=(t == cj - 1),
                            )
                        osl = slice(b * NO, (b + 1) * NO)
                        nc.vector.tensor_tensor(
                            acc[:, osl], acc[:, osl], pb[:],
                            op=mybir.AluOpType.add,
                        )
                        ci = cj

                    offL += nlo
                    offH += nhi
                    offD += nck

            # ---- final: scale by indeg^-1/2, add bias, write out ----
            ot = finpool.tile([P, NBLK * NO], F32, tag="out")
            for b in range(NBLK):
                osl = slice(b * NO, (b + 1) * NO)
                nc.vector.tensor_scalar_mul(ot[:, osl], acc[:, osl], isc[:, b:b + 1])
            nc.vector.tensor_tensor(
                ot[:].rearrange("p (b d) -> p b d", d=NO),
                ot[:].rearrange("p (b d) -> p b d", d=NO),
                bias_sb[:].rearrange("p (o d) -> p o d", o=1).broadcast_to(
                    [P, NBLK, NO]
                ),
                op=mybir.AluOpType.add,
            )
            nc.sync.dma_start(out.ap(), ot[:])

    nc.compile()
    return nc


def _wrap_stream(a):
    """flat [L] int array (L % 16 == 0) -> [128, L//16] int16 wrapped+replicated."""
    L = len(a)
    if L == 0:
        return np.zeros((P, 1), np.int16)
    w = a.reshape(L // 16, 16).T  # [16, L//16]
    return np.ascontiguousarray(np.tile(w, (8, 1)).astype(np.int16))


def _prep_host(feat, weight, bias, src, dst, cfg):
    NPAD, NCHUNK, SUP = cfg["NPAD"], cfg["NCHUNK"], cfg["SUP"]
    OWN, OWN_PAD, SPLIT = cfg["OWN"], cfg["OWN_PAD"], cfg["SPLIT"]
    GATES = cfg["GATES"]
    NF, NO, WS = cfg["NF"], cfg["NO"], cfg["WS"]
    NBLK = OWN_PAD // P
    n = feat.shape[0]
    ncore = cfg["NUM_DEV"]

    src = np.asarray(src)
    dst = np.asarray(dst)
    # rho: node u -> h row (u%128)*NCHUNK + u//128
    schunk = src // P               # source chunk 0..391 (gating)
    rho_src = (src % P).astype(np.int64) * NCHUNK + schunk
    ssup = schunk // SUP            # superblock 0..27

    outdeg = np.bincount(src, minlength=NPAD).astype(np.float32)
    outdeg[n:] = 1.0
    indeg = np.bincount(dst, minlength=ncore * OWN).astype(np.float32)

    featT_full = np.zeros((NF, NPAD), ml_dtypes.bfloat16)
    featT_full[:, :n] = np.asarray(feat, np.float32).T

    bias_t = np.ascontiguousarray(
        np.tile(np.asarray(bias, np.float32)[None, :], (P, 1))
    )
    weight_b = np.ascontiguousarray(
        np.asarray(weight, np.float32).astype(ml_dtypes.bfloat16)
    )
    iota_h = np.ascontiguousarray(
        np.tile(np.arange(P, dtype=np.float32)[None, :], (P, WS)).astype(
            ml_dtypes.bfloat16
        )
    )
    odeg_r = np.ascontiguousarray(outdeg.reshape(NCHUNK, P).T)

    core_of = dst // OWN
    # buckets[c][b] = (g_lo, d_lo, sup_lo, g_hi, d_hi, sup_hi), each sorted
    # by source chunk
    buckets = []
    for c in range(ncore):
        msk = core_of == c
        g = rho_src[msk]
        sup_e = ssup[msk]
        dl = (dst[msk] - c * OWN).astype(np.int64)
        blk = dl // P
        d128 = dl % P
        lo = g < SPLIT
        per_blk = []
        for b in range(NBLK):
            mb = blk == b
            mbl = mb & lo
            mbh = mb & ~lo
            ol = np.argsort(sup_e[mbl], kind="stable")
            oh = np.argsort(sup_e[mbh], kind="stable")
            per_blk.append(
                (
                    g[mbl][ol], d128[mbl][ol], sup_e[mbl][ol],
                    g[mbh][oh] - SPLIT, d128[mbh][oh], sup_e[mbh][oh],
                )
            )
        buckets.append(per_blk)

    CL = [0] * NBLK
    CH = [0] * NBLK
    for c in range(ncore):
        for b in range(NBLK):
            gl, _, _, gh, _, _ = buckets[c][b]
            CL[b] = max(CL[b], (len(gl) + P - 1) // P)
            CH[b] = max(CH[b], (len(gh) + P - 1) // P)

    # shared chunk gates: gate[b][k][j] = max over cores of (chunk max sup)+1
    gate = {}
    for b in range(NBLK):
        for k, CX in ((0, CL), (1, CH)):
            for j in range(CX[b]):
                gate[(b, k, j)] = 1  # pad chunks read row 0 (sup 0)
    for c in range(ncore):
        for b in range(NBLK):
            gl, _, sl, gh, _, sh = buckets[c][b]
            for k, sups in ((0, sl), (1, sh)):
                for j in range((len(sups) + P - 1) // P):
                    mx = int(sups[j * P: (j + 1) * P].max()) + 1
                    key = (b, k, j)
                    if mx > gate[key]:
                        gate[key] = mx

    # segment assignment by shared gate
    SEGOF = {}
    for key, gv in gate.items():
        for s_i, gb in enumerate(GATES):
            if gv <= gb:
                SEGOF[key] = s_i
                break
    cfg["CL"], cfg["CH"], cfg["SEGOF"] = CL, CH, SEGOF

    order, _ = _edge_layout(cfg)

    # per-core streams in processing order
    in_maps = []
    for c in range(ncore):
        # chunk-indexed views of this core's edges
        def chunk_data(b, k, j):
            gl, dvl, _, gh, dvh, _ = buckets[c][b]
            g, dvv = (gl, dvl) if k == 0 else (gh, dvh)
            gs = g[j * P:(j + 1) * P]
            ds = dvv[j * P:(j + 1) * P]
            # pad rows must be written by superblock 0 (gate 1): lo pad
            # -> row 0 (node 0); hi pad -> row 84*392 = 32928 (node 84),
            # the first hi-range row with source chunk 0.
            pad_idx = 0 if k == 0 else (84 * NCHUNK - SPLIT)
            gpad = np.full(P, pad_idx, np.int64)
            dpad = np.full(P, NO_MATCH, np.float32)
            gpad[: len(gs)] = gs
            dpad[: len(ds)] = ds
            return gpad, dpad

        gl_parts = []
        gh_parts = []
        dv_parts = []
        for s_i in range(len(GATES)):
            for (chunks, nlo, nhi) in order[s_i]:
                lo_chunks = [t for t in chunks if t[1] == 0]
                hi_chunks = [t for t in chunks if t[1] == 1]
                for b, k, j, pos in lo_chunks:
                    gp, _ = chunk_data(b, 0, j)
                    gl_parts.append(gp)
                for b, k, j, pos in hi_chunks:
                    gp, _ = chunk_data(b, 1, j)
                    gh_parts.append(gp)
                for b, k, j, pos in chunks:
                    _, dp = chunk_data(b, k, j)
                    dv_parts.append(dp)

        gl_stream = (
            np.concatenate(gl_parts) if gl_parts else np.zeros(0, np.int64)
        )
        gh_stream = (
            np.concatenate(gh_parts) if gh_parts else np.zeros(0, np.int64)
        )
        dvals = (
            np.stack(dv_parts) if dv_parts else np.zeros((0, P), np.float32)
        )

        ideg_c = np.full(OWN_PAD, 1.0, np.float32)
        ideg_c[:OWN] = indeg[c * OWN:(c + 1) * OWN]

        in_maps.append(
            {
                "featT": featT_full,
                "weight": weight_b,
                "bias_t": bias_t,
                "odeg": odeg_r,
                "ideg": np.ascontiguousarray(ideg_c.reshape(NBLK, P).T),
                "iota_h": iota_h,
                "gidxL": _wrap_stream(gl_stream),
                "gidxH": _wrap_stream(gh_stream),
                "dvalsT": np.ascontiguousarray(
                    dvals.T.astype(ml_dtypes.bfloat16)
                ),
            }
        )
    return in_maps


_NC_CACHE = {}


def _get_nc(cfg):
    key = (tuple(cfg["CL"]), tuple(cfg["CH"]),
           tuple(sorted(cfg["SEGOF"].items())))
    if key not in _NC_CACHE:
        _NC_CACHE[key] = build_nc(cfg)
    return _NC_CACHE[key]


def kernel(feat, weight, bias, src, dst, _trace=False, _trace_kwargs=None):
    cfg = _cfg_full()
    in_maps = _prep_host(feat, weight, bias, src, dst, cfg)
    nc = _get_nc(cfg)
    res = run_bass_kernel_spmd(
        nc,
        in_maps,
        core_ids=list(range(cfg["NUM_DEV"])),
        trace=_trace,
        **(_trace_kwargs or {}),
    )
    OWN, NBLK, NO = cfg["OWN"], cfg["OWN_PAD"] // P, cfg["NO"]
    outs = []
    for c in range(cfg["NUM_DEV"]):
        arr = res.results[c]["out"].reshape(P, NBLK, NO)
        own = arr.transpose(1, 0, 2).reshape(NBLK * P, NO)[:OWN]
        outs.append(own)
    out = np.ascontiguousarray(np.concatenate(outs, axis=0).astype(np.float32))
    if _trace:
        return out, res
    return out


# revision 32
# speedup vs baseline: 1.0923x; 1.0069x over previous
"""GCN GraphConv (norm='both') on 8 Trainium2 NeuronCores.

Strategy (V5, replicated projection + gated gather overlap):
  - Output rows (dst nodes) sharded across 8 cores: core c owns rows
    [c*6250, (c+1)*6250), viewed as 49 blocks of 128 dst rows.
  - Projection phase REPLICATED: every core computes the full
    h = (feat @ W) * outdeg^-1/2 table (50176 nodes, bf16, 256B rows in
    rho layout row = (n%128)*392 + n//128) into local DRAM, in 28
    superblocks of 14 chunks.  Each superblock's h-write DMA bumps a
    semaphore.  No collective, no cross-core sync at all.
  - Edge phase: per-core edges grouped by dst block; chunks of 128 edge
    slots, budgets (CL/CH per block) shared across cores so the program
    is SPMD-uniform.  Within each (block, lo/hi kind) the edges are
    sorted by source chunk, so each gather chunk has a monotone
    "gate" = number of h superblocks it needs.  Chunks are assigned to
    NSEG segments by cross-core-max gate; the gather stream waits on the
    h semaphore once per segment and otherwise runs concurrently with
    the projection phase (dma_gather's DRAM source is not
    dependency-tracked, so the manual sem is the only ordering).
  - Each chunk: dma_gather pulls h[rho(src)] rows into SBUF token
    layout; one-hot S[e,d] = (dstval[e] == d) built on DVE in bf16
    (batches of 8 chunks per instruction, physically-replicated iota to
    avoid broadcast APs); PE accumulates psum[128 dst, 64] += S.T @ G
    per chunk; per (block, segment) the psum partial is flushed into an
    SBUF f32 accumulator.
  - int16 gather-index limit handled by a lo/hi split at SPLIT=32768
    with a base-offset view h_all[SPLIT:].
  - Final per block: scale acc by indeg^-1/2, add bias, one dense DMA
    out (host un-permutes the [p, b] row order).

Host does integer-only graph preprocessing (degree counts, edge
bucketing, index remapping, gate computation).  bf16 is used for
feat/W/h (kernel-internal precision choice); accumulation is fp32 in
PSUM / SBUF.
"""

import sys

sys.path.insert(0, "/opt/trn_rl_repo")

import numpy as np
import ml_dtypes

import concourse.bacc as bacc
import concourse.bass as bass
import concourse.mybir as mybir
import concourse.tile as tile
from concourse.bass_utils import run_bass_kernel_spmd
from concourse.tile_rust import add_dep_helper

F32 = mybir.dt.float32
BF16 = mybir.dt.bfloat16
I16 = mybir.dt.int16
P = 128
NO_MATCH = 999.0  # dstval for pad slots; never equals iota 0..127

N_NODES = 50000
N_FEAT = 256
N_OUT = 64


def _cfg_full():
    return dict(
        NPAD=50176,          # padded node count (392 chunks of 128)
        NCHUNK=392,
        SUP=14,              # h chunks per projection superblock
        NSUP=28,
        OWN=6250,
        OWN_PAD=6272,        # 49 blocks of 128
        SPLIT=32768,
        BLK_G=7,             # dst blocks per group
        SEG=7,               # gather chunks per dma_gather instruction
        WS=8,                # one-hot chunks per DVE instruction
        GATES=(6, 12, 18, 24, 28),   # segment gate boundaries (superblocks)
        NF=N_FEAT,
        NO=N_OUT,
        NUM_DEV=8,
        CL=None,             # per-block lo chunk budgets (len 49)
        CH=None,             # per-block hi chunk budgets
        SEGOF=None,          # (b, kind, j) -> segment index
    )


def _edge_layout(cfg):
    """Shared (cross-core) processing order bookkeeping.

    Returns:
      order: list over segments of list over groups of
             list of (b, kind, j, stream_pos) in matmul order
             (block-major, lo chunks then hi chunks per block)
      lo_counts/hi_counts: per (seg, grp): number of lo / hi chunks
    """
    CL, CH, SEGOF = cfg["CL"], cfg["CH"], cfg["SEGOF"]
    NBLK = cfg["OWN_PAD"] // P
    NSEG = len(cfg["GATES"])
    BLK_G = cfg["BLK_G"]
    groups = [(g0, min(g0 + BLK_G, NBLK)) for g0 in range(0, NBLK, BLK_G)]
    order = []
    for s in range(NSEG):
        seg_groups = []
        for g0, g1 in groups:
            lo_pos = 0
            hi_pos = 0
            chunks = []
            for b in range(g0, g1):
                for j in range(CL[b]):
                    if SEGOF[(b, 0, j)] == s:
                        chunks.append((b, 0, j, lo_pos))
                        lo_pos += 1
                for j in range(CH[b]):
                    if SEGOF[(b, 1, j)] == s:
                        chunks.append((b, 1, j, hi_pos))
                        hi_pos += 1
            # reorder stream positions: lo chunks take 0..lo_pos-1 in
            # (b, j) order; hi likewise.  chunks[] already iterates in
            # that order so the pos values above are correct.
            seg_groups.append((chunks, lo_pos, hi_pos))
        order.append(seg_groups)
    return order, groups


def build_nc(cfg, debug=False):
    NPAD, NCHUNK = cfg["NPAD"], cfg["NCHUNK"]
    SUP, NSUP = cfg["SUP"], cfg["NSUP"]
    OWN_PAD, SPLIT = cfg["OWN_PAD"], cfg["SPLIT"]
    SEG, WS = cfg["SEG"], cfg["WS"]
    GATES = cfg["GATES"]
    NF, NO = cfg["NF"], cfg["NO"]
    CL, CH = cfg["CL"], cfg["CH"]
    NBLK = OWN_PAD // P
    KC = NF // P
    NSEG = len(GATES)
    assert SUP * NSUP == NCHUNK

    nc = bacc.Bacc(
        "TRN2",
        target_bir_lowering=False,
        debug=debug,
        num_devices=cfg["NUM_DEV"],
        num_swdge_queues=4,
        dynamic_dma_scratch_size=32768,
    )

    TL, TH = sum(CL) * P, sum(CH) * P
    TOTCK = sum(CL) + sum(CH)

    featT = nc.dram_tensor("featT", [NF, NPAD], BF16, kind="ExternalInput")
    weight = nc.dram_tensor("weight", [NF, NO], BF16, kind="ExternalInput")
    bias_t = nc.dram_tensor("bias_t", [P, NO], F32, kind="ExternalInput")
    odeg = nc.dram_tensor("odeg", [P, NCHUNK], F32, kind="ExternalInput")
    ideg = nc.dram_tensor("ideg", [P, NBLK], F32, kind="ExternalInput")
    iota_h = nc.dram_tensor("iota_h", [P, WS * P], BF16, kind="ExternalInput")
    gidxL = nc.dram_tensor("gidxL", [P, max(TL // 16, 1)], I16, kind="ExternalInput")
    gidxH = nc.dram_tensor("gidxH", [P, max(TH // 16, 1)], I16, kind="ExternalInput")
    dvalsT = nc.dram_tensor("dvalsT", [P, TOTCK], BF16, kind="ExternalInput")
    warm_idx = nc.dram_tensor("warm_idx", [P, 8], I16, kind="ExternalInput")

    out = nc.dram_tensor("out", [P, NBLK * NO], F32, kind="ExternalOutput")

    h_all = nc.dram_tensor("h_all", [NPAD, P], BF16)
    # rho layout: row = (n % 128) * NCHUNK + n // 128 -> [p, chunk, 128]
    h_view = h_all.ap().rearrange("(p c) d -> p c d", p=P)

    h_sem = nc.alloc_semaphore("h_ready")
    h_tok = nc.alloc_sbuf_tensor("h_tok", [P, NSUP * 2], BF16)
    h_tok2 = nc.alloc_sbuf_tensor("h_tok2", [P, NSUP * 2], BF16)

    order, groups = _edge_layout(cfg)

    with tile.TileContext(nc) as tc:
        with (
            tc.tile_pool(name="const", bufs=1) as cpool,
            tc.tile_pool(name="feat", bufs=3) as fpool,
            tc.tile_pool(name="hstage", bufs=2) as hpool,
            tc.tile_pool(name="psA", bufs=2, space="PSUM") as ppoolA,
            tc.tile_pool(name="psB", bufs=2, space="PSUM") as ppoolB,
            tc.tile_pool(name="gath", bufs=24) as gpool,
            tc.tile_pool(name="onehot", bufs=4) as spool,
            tc.tile_pool(name="fin", bufs=1) as finpool,
        ):
            # ---- constants ----
            w_sb = []
            for k in range(KC):
                wk = cpool.tile([P, NO], BF16, tag=f"w{k}")
                nc.sync.dma_start(wk[:], weight[k * P:(k + 1) * P, :])
                w_sb.append(wk)
            bias_sb = cpool.tile([P, NO], F32, tag="bias")
            nc.sync.dma_start(bias_sb[:], bias_t[:])
            iota_sb = cpool.tile([P, WS * P], BF16, tag="iota")
            nc.sync.dma_start(iota_sb[:], iota_h[:])

            osc = cpool.tile([P, NCHUNK], F32, tag="osc")
            nc.sync.dma_start(osc[:], odeg[:])
            nc.vector.tensor_scalar_max(osc[:], osc[:], 1.0)
            nc.scalar.activation(osc[:], osc[:], mybir.ActivationFunctionType.Sqrt)
            nc.vector.reciprocal(osc[:], osc[:])

            isc = cpool.tile([P, NBLK], F32, tag="isc")
            nc.sync.dma_start(isc[:], ideg[:])
            nc.vector.tensor_scalar_max(isc[:], isc[:], 1.0)
            nc.scalar.activation(isc[:], isc[:], mybir.ActivationFunctionType.Sqrt)
            nc.vector.reciprocal(isc[:], isc[:])

            # f32 accumulator for all 49 blocks; memset before gathers start
            acc = finpool.tile([P, NBLK * NO], F32, tag="acc")
            nc.vector.memset(acc[:], 0.0)

            # all gather indices + dst values loaded upfront into static
            # SBUF (no pool rotation -> no DMA-queue head blocking)
            gixA = cpool.tile([P, max(TL // 16, 1)], I16, tag="gixA")
            nc.sync.dma_start(gixA[:], gidxL[:])
            gixB = cpool.tile([P, max(TH // 16, 1)], I16, tag="gixB")
            nc.sync.dma_start(gixB[:], gidxH[:])
            dv_all = cpool.tile([P, TOTCK], BF16, tag="dva")
            nc.sync.dma_start(dv_all[:], dvalsT[:])

            gate_prev = [None]

            # ---- phase 1: replicated h = (feat @ W) * outdeg^-1/2 ----
            GW = 7  # chunks per wide psum group (448 f32 < 1 psum bank)
            assert SUP % GW == 0
            for s in range(NSUP):
                fts = []
                for k in range(KC):
                    ft = fpool.tile([P, SUP * P], BF16, tag=f"ft{k}")
                    nc.sync.dma_start(
                        ft[:],
                        featT[k * P:(k + 1) * P, s * SUP * P:(s + 1) * SUP * P],
                    )
                    fts.append(ft)
                hst = hpool.tile([P, SUP * P], BF16, tag="hst")
                hst3 = hst[:].rearrange("p (c d) -> p c d", d=P)
                # junk cols NO..P are never read downstream, but the
                # h-write DMA reads the whole tile (sim init check)
                nc.vector.memset(hst3[:, :, NO:], 0.0)
                for g0 in range(0, SUP, GW):
                    hp = ppoolA.tile([P, GW * NO], F32, tag="hp")
                    for cc in range(GW):
                        for k in range(KC):
                            nc.tensor.matmul(
                                hp[:, cc * NO:(cc + 1) * NO],
                                fts[k][:, (g0 + cc) * P:(g0 + cc + 1) * P],
                                w_sb[k][:],
                                start=(k == 0),
                                stop=(k == KC - 1),
                            )
                    c0 = s * SUP + g0
                    # one batched DVE op scales GW chunks at once
                    nc.vector.tensor_tensor(
                        hst3[:, g0:g0 + GW, :NO],
                        hp[:].rearrange("p (c d) -> p c d", d=NO),
                        osc[:, c0:c0 + GW].rearrange(
                            "p (c o) -> p c o", o=1
                        ).broadcast_to([P, GW, NO]),
                        op=mybir.AluOpType.mult,
                    )
                # h-write + token go on the Activation engine's HWDGE
                # queue so their waits never block the featT loads on the
                # sync queue.
                nc.scalar.dma_start(h_view[:, s * SUP:(s + 1) * SUP, :], hst[:])
                # token read-back: the h-write above IS range-tracked for
                # regular DMAs, so this tiny read waits for it; a vector
                # copy of the token then bumps h_sem (DMA instructions
                # have no free sem-update slot under TileContext).
                nc.scalar.dma_start(
                    h_tok[:, s * 2:(s + 1) * 2],
                    h_view[:, (s + 1) * SUP - 1:(s + 1) * SUP, 0:2].rearrange(
                        "p o d -> p (o d)"
                    ),
                )
                nc.scalar.activation(
                    h_tok2[:, s * 2:(s + 1) * 2],
                    h_tok[:, s * 2:(s + 1) * 2],
                    mybir.ActivationFunctionType.Copy,
                ).then_inc(h_sem, 16)

            # ---- edge phase: gated gathers + one-hot matmul reduce ----
            h_full = h_all.ap()
            h_hi = h_all.ap()[SPLIT:, :]

            # per-(seg, grp, kind) stream offsets into gidxL/gidxH/dvals
            offL = 0
            offH = 0
            offD = 0
            qcnt = [0]

            for s_i in range(NSEG):
                # anchor the gate wait between the previous segment's last
                # gather and this segment's gathers (nosync: same engine,
                # order-only) so the sem-optimizer cannot hoist it
                w = nc.gpsimd.wait_ge(h_sem, 16 * GATES[s_i])
                if gate_prev[0] is not None:
                    add_dep_helper(w.ins, gate_prev[0], sync=False,
                                   reason="segment gate order")
                for gi, (g0, g1) in enumerate(groups):
                    chunks, nlo, nhi = order[s_i][gi]
                    if not chunks:
                        continue
                    nck = len(chunks)

                    # gathers: per kind, split into <=SEG chunk instructions
                    tiles = ([], [])
                    for kind, (ck, gix, off, base_ap) in enumerate(
                        [(nlo, gixA, offL, h_full), (nhi, gixB, offH, h_hi)]
                    ):
                        for s0 in range(0, ck, SEG):
                            n = min(SEG, ck - s0)
                            gt = gpool.tile([P, SEG, P], BF16, tag="gt")
                            g_in = nc.gpsimd.dma_gather(
                                gt[:, :n, :],
                                base_ap,
                                gix[:, (off + s0) * 8:(off + s0 + n) * 8],
                                n * P,
                                n * P,
                                P,
                                queue_num=qcnt[0] % 4,
                            )
                            add_dep_helper(g_in.ins, w.ins, sync=False,
                                           reason="segment gate order")
                            gate_prev[0] = g_in.ins
                            qcnt[0] += 1
                            tiles[kind].append(gt)

                    # one-hot batches of WS chunks + per-chunk matmuls
                    # chunks[] is in matmul order; build S lazily per batch
                    sw_tiles = {}
                    for w0 in range(0, nck, WS):
                        wn = min(WS, nck - w0)
                        Sw = spool.tile([P, WS * P], BF16, tag="S")
                        nc.vector.tensor_tensor(
                            Sw[:, : wn * P].rearrange("p (w d) -> p w d", d=P),
                            iota_sb[:, : wn * P].rearrange("p (w d) -> p w d", d=P),
                            dv_all[:, offD + w0: offD + w0 + wn].rearrange(
                                "p (w o) -> p w o", o=1
                            ).broadcast_to([P, wn, P]),
                            op=mybir.AluOpType.is_equal,
                        )
                        sw_tiles[w0 // WS] = Sw

                    # matmuls: iterate blocks of this group in order
                    ci = 0
                    b_cur = -1
                    pb = None
                    while ci < nck:
                        b = chunks[ci][0]
                        # find extent of this block's chunks in this seg
                        cj = ci
                        while cj < nck and chunks[cj][0] == b:
                            cj += 1
                        pb = ppoolB.tile([P, NO], F32, tag="pb")
                        for t in range(ci, cj):
                            _, kind, j, pos = chunks[t]
                            gt = tiles[kind][pos // SEG]
                            Sw = sw_tiles[t // WS]
                            nc.tensor.matmul(
                                pb[:],
                                Sw[:, (t % WS) * P:(t % WS + 1) * P],
                                gt[:, pos % SEG, :NO],
                                start=(t == ci),
                                stop=(t == cj - 1),
                            )
                        osl = slice(b * NO, (b + 1) * NO)
                        nc.vector.tensor_tensor(
                            acc[:, osl], acc[:, osl], pb[:],
                            op=mybir.AluOpType.add,
                        )
                        ci = cj

                    offL += nlo
                    offH += nhi
                    offD += nck

            # ---- final: scale by indeg^-1/2, add bias, write out ----
            ot = finpool.tile([P, NBLK * NO], F32, tag="out")
            for b in range(NBLK):
                osl = slice(b * NO, (b + 1) * NO)
                nc.vector.tensor_scalar_mul(ot[:, osl], acc[:, osl], isc[:, b:b + 1])
            nc.vector.tensor_tensor(
                ot[:].rearrange("p (b d) -> p b d", d=NO),
                ot[:].rearrange("p (b d) -> p b d", d=NO),
                bias_sb[:].rearrange("p (o d) -> p o d", o=1).broadcast_to(
                    [P, NBLK, NO]
                ),
                op=mybir.AluOpType.add,
            )
            nc.sync.dma_start(out.ap(), ot[:])

    nc.compile()
    return nc


def _wrap_stream(a):
    """flat [L] int array (L % 16 == 0) -> [128, L//16] int16 wrapped+replicated."""
    L = len(a)
    if L == 0:
        return np.zeros((P, 1), np.int16)
    w = a.reshape(L // 16, 16).T  # [16, L//16]
    return np.ascontiguousarray(np.tile(w, (8, 1)).astype(np.int16))


def _prep_host(feat, weight, bias, src, dst, cfg):
    NPAD, NCHUNK, SUP = cfg["NPAD"], cfg["NCHUNK"], cfg["SUP"]
    OWN, OWN_PAD, SPLIT = cfg["OWN"], cfg["OWN_PAD"], cfg["SPLIT"]
    GATES = cfg["GATES"]
    NF, NO, WS = cfg["NF"], cfg["NO"], cfg["WS"]
    NBLK = OWN_PAD // P
    n = feat.shape[0]
    ncore = cfg["NUM_DEV"]

    src = np.asarray(src)
    dst = np.asarray(dst)
    # rho: node u -> h row (u%128)*NCHUNK + u//128
    schunk = src // P               # source chunk 0..391 (gating)
    rho_src = (src % P).astype(np.int64) * NCHUNK + schunk
    ssup = schunk // SUP            # superblock 0..27

    outdeg = np.bincount(src, minlength=NPAD).astype(np.float32)
    outdeg[n:] = 1.0
    indeg = np.bincount(dst, minlength=ncore * OWN).astype(np.float32)

    featT_full = np.zeros((NF, NPAD), ml_dtypes.bfloat16)
    featT_full[:, :n] = np.asarray(feat, np.float32).T

    bias_t = np.ascontiguousarray(
        np.tile(np.asarray(bias, np.float32)[None, :], (P, 1))
    )
    weight_b = np.ascontiguousarray(
        np.asarray(weight, np.float32).astype(ml_dtypes.bfloat16)
    )
    iota_h = np.ascontiguousarray(
        np.tile(np.arange(P, dtype=np.float32)[None, :], (P, WS)).astype(
            ml_dtypes.bfloat16
        )
    )
    odeg_r = np.ascontiguousarray(outdeg.reshape(NCHUNK, P).T)

    core_of = dst // OWN
    # buckets[c][b] = (g_lo, d_lo, sup_lo, g_hi, d_hi, sup_hi), each sorted
    # by source chunk
    buckets = []
    for c in range(ncore):
        msk = core_of == c
        g = rho_src[msk]
        sup_e = ssup[msk]
        dl = (dst[msk] - c * OWN).astype(np.int64)
        blk = dl // P
        d128 = dl % P
        lo = g < SPLIT
        per_blk = []
        for b in range(NBLK):
            mb = blk == b
            mbl = mb & lo
            mbh = mb & ~lo
            ol = np.argsort(sup_e[mbl], kind="stable")
            oh = np.argsort(sup_e[mbh], kind="stable")
            per_blk.append(
                (
                    g[mbl][ol], d128[mbl][ol], sup_e[mbl][ol],
                    g[mbh][oh] - SPLIT, d128[mbh][oh], sup_e[mbh][oh],
                )
            )
        buckets.append(per_blk)

    CL = [0] * NBLK
    CH = [0] * NBLK
    for c in range(ncore):
        for b in range(NBLK):
            gl, _, _, gh, _, _ = buckets[c][b]
            CL[b] = max(CL[b], (len(gl) + P - 1) // P)
            CH[b] = max(CH[b], (len(gh) + P - 1) // P)

    # shared chunk gates: gate[b][k][j] = max over cores of (chunk max sup)+1
    gate = {}
    for b in range(NBLK):
        for k, CX in ((0, CL), (1, CH)):
            for j in range(CX[b]):
                gate[(b, k, j)] = 1  # pad chunks read row 0 (sup 0)
    for c in range(ncore):
        for b in range(NBLK):
            gl, _, sl, gh, _, sh = buckets[c][b]
            for k, sups in ((0, sl), (1, sh)):
                for j in range((len(sups) + P - 1) // P):
                    mx = int(sups[j * P: (j + 1) * P].max()) + 1
                    key = (b, k, j)
                    if mx > gate[key]:
                        gate[key] = mx

    # segment assignment by shared gate
    SEGOF = {}
    for key, gv in gate.items():
        for s_i, gb in enumerate(GATES):
            if gv <= gb:
                SEGOF[key] = s_i
                break
    cfg["CL"], cfg["CH"], cfg["SEGOF"] = CL, CH, SEGOF

    order, _ = _edge_layout(cfg)

    # per-core streams in processing order
    in_maps = []
    for c in range(ncore):
        # chunk-indexed views of this core's edges
        def chunk_data(b, k, j):
            gl, dvl, _, gh, dvh, _ = buckets[c][b]
            g, dvv = (gl, dvl) if k == 0 else (gh, dvh)
            gs = g[j * P:(j + 1) * P]
            ds = dvv[j * P:(j + 1) * P]
            # pad rows must be written by superblock 0 (gate 1): lo pad
            # -> row 0 (node 0); hi pad -> row 84*392 = 32928 (node 84),
            # the first hi-range row with source chunk 0.
            pad_idx = 0 if k == 0 else (84 * NCHUNK - SPLIT)
            gpad = np.full(P, pad_idx, np.int64)
            dpad = np.full(P, NO_MATCH, np.float32)
            gpad[: len(gs)] = gs
            dpad[: len(ds)] = ds
            return gpad, dpad

        gl_parts = []
        gh_parts = []
        dv_parts = []
        for s_i in range(len(GATES)):
            for (chunks, nlo, nhi) in order[s_i]:
                lo_chunks = [t for t in chunks if t[1] == 0]
                hi_chunks = [t for t in chunks if t[1] == 1]
                for b, k, j, pos in lo_chunks:
                    gp, _ = chunk_data(b, 0, j)
                    gl_parts.append(gp)
                for b, k, j, pos in hi_chunks:
                    gp, _ = chunk_data(b, 1, j)
                    gh_parts.append(gp)
                for b, k, j, pos in chunks:
                    _, dp = chunk_data(b, k, j)
                    dv_parts.append(dp)

        gl_stream = (
            np.concatenate(gl_parts) if gl_parts else np.zeros(0, np.int64)
        )
        gh_stream = (
            np.concatenate(gh_parts) if gh_parts else np.zeros(0, np.int64)
        )
        dvals = (
            np.stack(dv_parts) if dv_parts else np.zeros((0, P), np.float32)
        )

        ideg_c = np.full(OWN_PAD, 1.0, np.float32)
        ideg_c[:OWN] = indeg[c * OWN:(c + 1) * OWN]

        in_maps.append(
            {
                "featT": featT_full,
                "weight": weight_b,
                "bias_t": bias_t,
                "odeg": odeg_r,
                "ideg": np.ascontiguousarray(ideg_c.reshape(NBLK, P).T),
                "iota_h": iota_h,
                "gidxL": _wrap_stream(gl_stream),
                "gidxH": _wrap_stream(gh_stream),
                "dvalsT": np.ascontiguousarray(
                    dvals.T.astype(ml_dtypes.bfloat16)
                ),
                "warm_idx": np.zeros((P, 8), np.int16),
            }
        )
    return in_maps


_NC_CACHE = {}


def _get_nc(cfg):
    key = (tuple(cfg["CL"]), tuple(cfg["CH"]),
           tuple(sorted(cfg["SEGOF"].items())))
    if key not in _NC_CACHE:
        _NC_CACHE[key] = build_nc(cfg)
    return _NC_CACHE[key]


def kernel(feat, weight, bias, src, dst, _trace=False, _trace_kwargs=None):
    cfg = _cfg_full()
    in_maps = _prep_host(feat, weight, bias, src, dst, cfg)
    nc = _get_nc(cfg)
    res = run_bass_kernel_spmd(
        nc,
        in_maps,
        core_ids=list(range(cfg["NUM_DEV"])),
        trace=_trace,
        **(_trace_kwargs or {}),
    )
    OWN, NBLK, NO = cfg["OWN"], cfg["OWN_PAD"] // P, cfg["NO"]
    outs = []
    for c in range(cfg["NUM_DEV"]):
        arr = res.results[c]["out"].reshape(P, NBLK, NO)
        own = arr.transpose(1, 0, 2).reshape(NBLK * P, NO)[:OWN]
        outs.append(own)
    out = np.ascontiguousarray(np.concatenate(outs, axis=0).astype(np.float32))
    if _trace:
        return out, res
    return out
